# revision 1
# baseline (speedup 1.0000x reference)
"""Trainium2 Bass kernel for nn_DADPolicy (GNN pooling + LSTM + pair scorer).

Math (see reference):
  hn = mean_relu(node_feats @ node_W + node_b)           (64,)
  he = mean_relu(edge_feats @ edge_W + edge_b)           (64,)
  z_hist = LSTM(hist_tokens)                             (64,)
  c0 = [hn, he, z_hist] @ fuse_W1[:192] + fuse_b1        (64,)
  h_p = relu(c0 + u_p*W1u + v_p*W1v);  score_p = h_p @ fuse_W2 + fuse_b2

Sharding: data parallel over 8 cores (nodes/edges/pairs sharded, LSTM +
weights replicated, one [128,2] AllReduce for the pooled sums).

Device structure:
  - Encoders: block-diagonal lhsT packs two row-groups into K (features of
    2 groups on partitions, 32-aligned blocks), M = 2x64 hidden; relu +
    row-sum + bias fused into one ACT/DVE drain op per psum tile
    (accum_out), alternating engines.
  - LSTM: tanh-only gates (sigmoid(x) = (tanh(x/2)+1)/2); gate preacts via
    4 small matmuls against rhs = [2h ; x_t ; 1] with all scales folded
    into weights, one tanh ACT per step over the [64,4] gate tile.
  - Pair scorer: mm1 = diag(W1u_v) [4,128] -> [2x64 hidden, pairs]; relu
    drain adds c0 via the bias operand and emits bf16; mm2 swaps operands
    (S chunk stationary [128,128] bf16, rhs = [W2;0 | 0;W2]) so scores
    land as [128 pairs, 2 groups] psum stacked along free dim.
"""

import os

import numpy as np

import concourse.bass as bass
import concourse.mybir as mybir
import concourse.tile as tile
from concourse.bass_utils import run_bass_kernel_spmd
from concourse.vector_clock import ScopedClock

f32 = mybir.dt.float32
f16 = mybir.dt.float16
bf16 = mybir.dt.bfloat16
AF = mybir.ActivationFunctionType
ALU = mybir.AluOpType

H = 64
NCORES = 8

# ---- edge geometry (per core) ----
# 12 row-groups; 32-partition block B holds groups 4B..4B+3 as rows
# (5*gl + f) at partitions 32B..32B+20 (rows 20-31 of each block pad).
E_TOT = 3200000
EPC = E_TOT // NCORES            # 400000
EGROUPS = 12
EGCOLS = 33792                   # padded rows per group (= 66*512 = 6*5632)
EPAD_ROWS = EGROUPS * EGCOLS     # 405504
ETILE = 5632                     # cols per DMA tile (11*512)

# ---- node geometry (per core) ----
N_TOT = 100000
NPC = N_TOT // NCORES            # 12500
NGCOLS = 6656                    # 13*512, 2 groups
NPAD_ROWS = 2 * NGCOLS           # 13312

# ---- pair geometry (per core) ----
P_TOT = 1000000
PPC = P_TOT // NCORES            # 125000
PGROUPS = 6
PGCOLS = 21504                   # 42*512 = 21*1024
PPAD = PGROUPS * PGCOLS          # 129024

T_HIST = 200
SC_COLS = 16 * 63                # 1008 score columns
CAND_C = 49999.5                 # host-side centering of pair indices
CAND_SCALE = 4096.0              # fp16 range scaling: idx -> (idx-C)/SCALE

# The walrus in this container rejects instructions carrying more than a
# couple of semaphore waits ("Too many sync wait commands" in
# CoreV3GenImpl setupSyncWait). Tile freely aggregates waits onto one
# instruction. Post-pass: split excess waits onto fresh single-wait NOPs
# inserted immediately before the overflowing instruction (same engine,
# same program position -> semantics unchanged).
import bass_rust as _br

_WAIT_LIMITS = {"InstDMACopy": 1, "InstDrain": 1, "InstNoOp": 1,
                "InstCollectiveCompute": 1}
_WAIT_LIMIT_DEFAULT = 1


def _split_excess_waits(nc):
    fn = nc.m.functions[0]
    n_split = 0
    for bb in fn.blocks:
        insts = bb.instructions
        i = 0
        while i < len(insts):
            ins = insts[i]
            si = ins.sync_info
            lim = _WAIT_LIMITS.get(type(ins).__name__, _WAIT_LIMIT_DEFAULT)
            if si is not None and si.on_wait and len(si.on_wait) > lim:
                waits = list(si.on_wait)
                si.on_wait = waits[:lim]
                for w in waits[lim:]:
                    nop = mybir.InstNoOp(
                        name=nc.get_next_instruction_name(), ins=[], outs=[]
                    )
                    nop.engine = ins.engine
                    nop.sync_info = _br.SyncInfo(on_wait=[w], on_update=[])
                    nc.register_instruction(nop)
                    insts.insert(i, nop)
                    i += 1
                    n_split += 1
            i += 1
    print(f"split_excess_waits: inserted {n_split} wait-nops")
    return nc


def build_nc():
    nc = bass.Bass(num_devices=NCORES)
    tc = tile.TileContext(nc)

    def inp(name, shape, dt=f32):
        return nc.declare_dram_parameter(name, list(shape), dt, isOutput=False)

    edgeT = inp("edgeT", (96, EGCOLS), f16)
    nodeT = inp("nodeT", (4, NGCOLS), f16)
    candT = inp("candT", (12, PGCOLS), f16)
    hist = inp("hist", (T_HIST, 3))
    # per 32-block: [20, 0:128] = block-diag4 of edge_W[:, :32] (lo),
    #               [20, 128:256] = same for edge_W[:, 32:] (hi)
    lhsT_e = inp("lhsT_e", (84, 256), f16)
    lhsT_n = inp("lhsT_n", (4, 128), f16)              # diag2(node_W)
    lhsT_p1 = inp("lhsT_p1", (68, 128), f16)           # diag2(W1u_v) @0/32/64
    w2stack = inp("w2stack", (128, 2), f16)            # [W2;0 | 0;W2]
    lhsT_g4 = inp("lhsT_g4", (68, 4 * H))              # lstm gate blocks
    w1a_d = inp("w1a_d", (128, 128))                   # tile2x2(W1a/N)
    w1b_lo = inp("w1b_lo", (128, 128))                 # tile(W1b[:32]/E,(4,2))
    w1b_hi = inp("w1b_hi", (128, 128))                 # tile(W1b[32:]/E,(4,2))
    w1h_d = inp("w1h_d", (H, 128))                     # tile(W1h/2, (1,2))
    # 0: edge_b[p%32] 1: edge_b[32+p%32] 2: [node_b;node_b] 3: b1adj
    # 4: b2 5: 0.5
    cvec = inp("cvec", (128, 8))

    out_scores = nc.declare_dram_parameter(
        "scores", [128, SC_COLS], f32, isOutput=True
    )
    out_dbg = nc.declare_dram_parameter("dbg", [128, 4], f32, isOutput=True)

    cc_in = nc.dram_tensor("cc_in", [128, 4], f32)
    cc_out = nc.dram_tensor("cc_out", [128, 4], f32)

    with tc:
        with (
            tc.tile_pool(name="consts", bufs=1) as const_pool,
            tc.tile_pool(name="state", bufs=1) as state_pool,
            tc.tile_pool(name="small", bufs=4) as small_pool,
            tc.tile_pool(name="lstm_psum", bufs=2, space="PSUM") as lstm_psum,
        ):
            # ---------------- constants ----------------
            def ld(tag, shape, ap, dt=f32):
                t = const_pool.tile(list(shape), dt, tag=tag)
                nc.sync.dma_start(out=t[:, :], in_=ap)
                return t

            c_lhsT_e = ld("c_lhsT_e", (84, 256), lhsT_e[:, :], f16)
            c_lhsT_n = ld("c_lhsT_n", (4, 128), lhsT_n[:, :], f16)
            c_lhsT_p1 = ld("c_lhsT_p1", (68, 128), lhsT_p1[:, :], f16)
            c_w2b = ld("c_w2b", (128, 2), w2stack[:, :], f16)
            c_g4 = ld("c_g4", (68, 4 * H), lhsT_g4[:, :])
            c_w1a = ld("c_w1a", (128, 128), w1a_d[:, :])
            c_w1b_lo = ld("c_w1b_lo", (128, 128), w1b_lo[:, :])
            c_w1b_hi = ld("c_w1b_hi", (128, 128), w1b_hi[:, :])
            c_w1h = ld("c_w1h", (H, 128), w1h_d[:, :])
            c_cv = ld("c_cv", (128, 8), cvec[:, :])
            zeros_t = const_pool.tile([128, 1536], f32, tag="zeros")
            nc.vector.memset(zeros_t[:, :], 0.0)

            bias_e_lo = c_cv[:, 0:1]
            bias_e_hi = c_cv[:, 1:2]
            bias_n = c_cv[:, 2:3]
            b1adj = c_cv[:, 3:4]
            b2col = c_cv[:, 4:5]
            half64 = c_cv[0:H, 5:6]

            # ---------------- persistent state ----------------
            acc_e = state_pool.tile([128, 160], f32)  # lo slots 0:80, hi 80:160
            acc_n = state_pool.tile([128, 8], f32)
            nc.vector.memset(acc_e[:, :], 0.0)
            nc.vector.memset(acc_n[:, :], 0.0)
            cst = state_pool.tile([H, 1], f32)
            nc.vector.memset(cst[:, :], 0.0)
            # histX column t = [h2_{t-1}(64) ; x_t(3) ; 1]
            histX = state_pool.tile([68, T_HIST + 1], f32)
            nc.vector.memset(histX[0:H, :], 0.0)
            nc.vector.memset(histX[H:68, :], 1.0)
            nc.sync.dma_start(
                out=histX[H:H + 3, 0:T_HIST],
                in_=hist[:, :].rearrange("t f -> f t"),
            )
            pools_v = state_pool.tile([128, 4], f32)
            nc.vector.memset(pools_v[:, :], 0.0)
            pools_r = state_pool.tile([128, 4], f32)
            c0_stack = state_pool.tile([128, 1], f32)
            scores_sb = state_pool.tile([128, SC_COLS], f32)

            nsb = state_pool.tile([4, NGCOLS], f16)
            nc.sync.dma_start(out=nsb[:, :], in_=nodeT[:, :])

            csb = state_pool.tile([68, PGCOLS], f16)

            # ---------------- LSTM ----------------
            for t in range(T_HIST):
                g_ps = lstm_psum.tile([H, 4], f32, tag="g")
                for gi_ in range(4):
                    nc.tensor.matmul(
                        g_ps[:, gi_:gi_ + 1],
                        c_g4[:, H * gi_:H * (gi_ + 1)],
                        histX[:, t:t + 1],
                        start=True, stop=True,
                    )
                T4 = small_pool.tile([H, 4], f32, tag="T4")
                nc.scalar.activation(T4[:, :], g_ps[:, :], AF.Tanh)
                u = small_pool.tile([H, 1], f32, tag="u")
                v = small_pool.tile([H, 1], f32, tag="v")
                # u = (c * Tf) + c = 2*sig(f)*c
                nc.vector.scalar_tensor_tensor(
                    u[:, :], cst[:, :], T4[:, 1:2], cst[:, :],
                    op0=ALU.mult, op1=ALU.add,
                )
                # v = (Tg * Ti) + Tg = 2*sig(i)*tanh(g)
                nc.vector.scalar_tensor_tensor(
                    v[:, :], T4[:, 2:3], T4[:, 0:1], T4[:, 2:3],
                    op0=ALU.mult, op1=ALU.add,
                )
                # c = (u + v) * 0.5
                nc.vector.scalar_tensor_tensor(
                    cst[:, :], u[:, :], v[:, :], half64,
                    op0=ALU.add, op1=ALU.mult,
                )
                tC = small_pool.tile([H, 1], f32, tag="tC")
                nc.scalar.activation(tC[:, :], cst[:, :], AF.Tanh)
                # h2_t = (tC * To) + tC = 2*sig(o)*tanh(c)
                nc.vector.scalar_tensor_tensor(
                    histX[0:H, t + 1:t + 2], tC[:, :], T4[:, 3:4], tC[:, :],
                    op0=ALU.mult, op1=ALU.add,
                )

            # ---------------- edge + node encoders ----------------
            with (
                tc.tile_pool(name="edgesb", bufs=3) as edge_pool,
                tc.tile_pool(name="enc_psum", bufs=2, space="PSUM") as enc_psum,
            ):
                drain_i = 0

                def drain(ps_ap, bias_ap, slot_ap):
                    # relu(psum + bias) summed along free dim into slot
                    nonlocal drain_i
                    if drain_i % 2 == 0:
                        nc.scalar.activation(
                            ps_ap, ps_ap, AF.Relu, bias=bias_ap,
                            accum_out=slot_ap,
                        )
                    else:
                        n = ps_ap.shape[-1]
                        nc.vector.scalar_tensor_tensor(
                            ps_ap, ps_ap, bias_ap, zeros_t[:, 0:n],
                            op0=ALU.add, op1=ALU.max, accum_out=slot_ap,
                        )
                    drain_i += 1

                # nodes: single K=4 block at partition 0
                ci, sloti = 0, 0
                while ci < 13:
                    take = min(3, 13 - ci)
                    ps = enc_psum.tile([128, 1536], f32, tag="big")
                    for k in range(take):
                        nc.tensor.matmul(
                            ps[:, 512 * k:512 * (k + 1)],
                            c_lhsT_n[:, :],
                            nsb[:, 512 * (ci + k):512 * (ci + k + 1)],
                            start=True, stop=True,
                        )
                    drain(ps[:, 0:512 * take], bias_n, acc_n[:, sloti:sloti + 1])
                    ci += take
                    sloti += 1

                # edges: 6 dense DMA tiles [96, 5632]; per tile: 3 blocks x
                # 2 hidden-halves x 11 col-chunks of 512
                for d in range(6):
                    esb = edge_pool.tile([96, ETILE], f16, tag="esb")
                    nc.sync.dma_start(
                        out=esb[:, :],
                        in_=edgeT[:, ETILE * d:ETILE * (d + 1)],
                    )
                    for B in range(3):
                        for hf in range(2):
                            ci, dg = 0, 0
                            while ci < 11:
                                take = min(3, 11 - ci)
                                ps = enc_psum.tile([128, 1536], f32, tag="big")
                                for k in range(take):
                                    c = ci + k
                                    nc.tensor.matmul(
                                        ps[:, 512 * k:512 * (k + 1)],
                                        c_lhsT_e[32 * B:32 * B + 20,
                                                 128 * hf:128 * (hf + 1)],
                                        esb[32 * B:32 * B + 20,
                                            512 * c:512 * (c + 1)],
                                        start=True, stop=True,
                                    )
                                slot = 80 * hf + (d * 3 + B) * 4 + dg
                                drain(ps[:, 0:512 * take],
                                      bias_e_hi if hf else bias_e_lo,
                                      acc_e[:, slot:slot + 1])
                                ci += take
                                dg += 1

                nc.vector.tensor_reduce(
                    pools_v[:, 1:2], acc_e[:, 0:80], axis=mybir.AxisListType.X,
                    op=ALU.add,
                )
                nc.vector.tensor_reduce(
                    pools_v[:, 2:3], acc_e[:, 80:160],
                    axis=mybir.AxisListType.X, op=ALU.add,
                )
                nc.vector.tensor_reduce(
                    pools_v[:, 0:1], acc_n[:, :], axis=mybir.AxisListType.X,
                    op=ALU.add,
                )

            # prefetch pair candidates (needed only after the collective)
            for B in range(3):
                nc.scalar.dma_start(
                    out=csb[32 * B:32 * B + 4, :], in_=candT[4 * B:4 * B + 4, :]
                )

            # ---------------- all-reduce pooled sums ----------------
            nc.gpsimd.dma_start(out=cc_in[:, :], in_=pools_v[:, :])
            nc.gpsimd.collective_compute(
                "AllReduce", ALU.add,
                replica_groups=[list(range(NCORES))],
                ins=[cc_in[:, :]],
                outs=[cc_out[:, :]],
            )
            nc.gpsimd.dma_start(out=pools_r[:, :], in_=cc_out[:, :])

            # ---------------- c0 context vector ----------------
            c0_ps = lstm_psum.tile([128, 1], f32, tag="g")
            nc.tensor.matmul(c0_ps[:, 0:1], c_w1a[:, :], pools_r[:, 0:1],
                             start=True, stop=False)
            nc.tensor.matmul(c0_ps[:, 0:1], c_w1b_lo[:, :], pools_r[:, 1:2],
                             start=False, stop=False)
            nc.tensor.matmul(c0_ps[:, 0:1], c_w1b_hi[:, :], pools_r[:, 2:3],
                             start=False, stop=False)
            nc.tensor.matmul(c0_ps[:, 0:1], c_w1h[:, :],
                             histX[0:H, T_HIST:T_HIST + 1],
                             start=False, stop=True)
            nc.scalar.activation(c0_stack[:, :], c0_ps[:, 0:1], AF.Identity,
                                 bias=b1adj)

            # ---------------- pair scorer ----------------
            with (
                tc.tile_pool(name="s_pool", bufs=3) as s_pool,
                tc.tile_pool(name="pair_psum", bufs=2, space="PSUM") as pair_psum,
                tc.tile_pool(name="sc_psum", bufs=1, space="PSUM") as sc_psum,
            ):
                scps = sc_psum.tile([128, SC_COLS], f32)
                di = 0
                for B in range(3):
                    for cc in range(21):
                        ps = pair_psum.tile([128, 1024], f32, tag="p")
                        for i in range(2):
                            c = 2 * cc + i
                            nc.tensor.matmul(
                                ps[:, 512 * i:512 * (i + 1)],
                                c_lhsT_p1[32 * B:32 * B + 4, :],
                                csb[32 * B:32 * B + 4,
                                    512 * c:512 * (c + 1)],
                                start=True, stop=True,
                            )
                        s_t = s_pool.tile([128, 1024], f16, tag="s")
                        if di % 2 == 0:
                            nc.scalar.activation(
                                s_t[:, :], ps[:, :], AF.Relu,
                                bias=c0_stack[:, 0:1],
                            )
                        else:
                            nc.vector.tensor_scalar(
                                s_t[:, :], ps[:, :], c0_stack[:, 0:1], 0.0,
                                op0=ALU.add, op1=ALU.max,
                            )
                        di += 1
                        j = 21 * B + cc
                        for m in range(8):
                            nc.tensor.matmul(
                                scps[:, 16 * j + 2 * m:16 * j + 2 * m + 2],
                                s_t[:, 128 * m:128 * (m + 1)],
                                c_w2b[:, :],
                                start=True, stop=True,
                            )

                nc.scalar.activation(
                    scores_sb[:, :], scps[:, :], AF.Identity, bias=b2col,
                )
                nc.sync.dma_start(out=out_scores[:, :], in_=scores_sb[:, :])
                nc.sync.dma_start(out=out_dbg[:, 0:2], in_=pools_r[:, 0:2])
                nc.sync.dma_start(out=out_dbg[:, 2:3], in_=c0_stack[:, :])
                nc.sync.dma_start(out=out_dbg[0:H, 3:4],
                                  in_=histX[0:H, T_HIST:T_HIST + 1])

    return _split_excess_waits(nc)


# ======================= host side =======================

def _prep_weights(node_W, node_b, edge_W, edge_b,
                  lstm_Wih, lstm_Whh, lstm_bih, lstm_bhh,
                  fuse_W1, fuse_b1, fuse_W2, fuse_b2,
                  denom, n_zero_node, n_zero_edge):
    f = np.float32

    def diag2(W):  # W (k, 64) -> (2k, 128) block diagonal
        k = W.shape[0]
        out = np.zeros((2 * k, 128), f)
        out[:k, :H] = W
        out[k:, H:] = W
        return out

    # edge lhsT: per 32-block, rows (5*gl+f), cols [lo | hi] hidden halves
    lhsT_e = np.zeros((84, 256), np.float16)
    blk = np.zeros((20, 256), f)
    for gl in range(4):
        blk[5 * gl:5 * gl + 5, 32 * gl:32 * gl + 32] = edge_W[:, 0:32]
        blk[5 * gl:5 * gl + 5, 128 + 32 * gl:128 + 32 * gl + 32] = edge_W[:, 32:64]
    for B in range(3):
        lhsT_e[32 * B:32 * B + 20] = blk
    lhsT_n = diag2(node_W.astype(f)).astype(np.float16)
    W1u_v = fuse_W1[192:194].astype(np.float64) / denom * CAND_SCALE
    lhsT_p1 = np.zeros((68, 128), np.float16)
    for B in range(3):
        lhsT_p1[32 * B:32 * B + 4] = diag2(W1u_v.astype(f))
    w2stack = np.zeros((128, 2), np.float16)
    w2stack[:H, 0] = fuse_W2[:, 0]
    w2stack[H:, 1] = fuse_W2[:, 0]

    # lstm gate blocks; sigmoid gates folded to tanh(x/2), h2 = 2*h stored
    Wih = lstm_Wih.astype(np.float64)
    Whh = lstm_Whh.astype(np.float64)
    bc = (lstm_bih + lstm_bhh).astype(np.float64)
    lhsT_g4 = np.zeros((68, 4 * H), f)
    for k, (g0, sg) in enumerate(
        [(0, 0.5), (H, 0.5), (2 * H, 1.0), (3 * H, 0.5)]
    ):
        lhsT_g4[0:H, k * H:(k + 1) * H] = (sg * 0.5 * Whh[g0:g0 + H]).T
        lhsT_g4[H:H + 3, k * H:(k + 1) * H] = (sg * Wih[g0:g0 + H]).T
        lhsT_g4[67, k * H:(k + 1) * H] = sg * bc[g0:g0 + H]

    W1a = fuse_W1[0:H].astype(np.float64) / N_TOT
    W1b = fuse_W1[H:2 * H].astype(np.float64) / E_TOT
    W1h = fuse_W1[2 * H:3 * H].astype(np.float64) / 2.0
    w1a_d = np.tile(W1a, (2, 2)).astype(f)
    w1b_lo = np.tile(W1b[0:32], (4, 2)).astype(f)
    w1b_hi = np.tile(W1b[32:64], (4, 2)).astype(f)
    w1h_d = np.tile(W1h, (1, 2)).astype(f)

    relu = lambda x: np.maximum(x, 0.0)
    b1 = fuse_b1.astype(np.float64).copy()
    # candidate indices are centered by CAND_C on the host; fold the
    # constant part of (u, v) @ W1u_v back into the bias
    b1 += (CAND_C / denom) * (fuse_W1[192].astype(np.float64)
                              + fuse_W1[193].astype(np.float64))
    b1 -= n_zero_node * relu(node_b.astype(np.float64)) @ (
        fuse_W1[0:H].astype(np.float64) / N_TOT)
    b1 -= n_zero_edge * relu(edge_b.astype(np.float64)) @ (
        fuse_W1[H:2 * H].astype(np.float64) / E_TOT)
    b1adj = np.tile(b1.astype(f), 2)

    cvec = np.zeros((128, 8), f)
    cvec[:, 0] = np.tile(edge_b.astype(f)[0:32], 4)
    cvec[:, 1] = np.tile(edge_b.astype(f)[32:64], 4)
    cvec[:, 2] = np.tile(node_b.astype(f), 2)
    cvec[:, 3] = b1adj
    cvec[:, 4] = np.float32(fuse_b2[0])
    cvec[:, 5] = 0.5

    return dict(
        lhsT_e=lhsT_e, lhsT_n=lhsT_n, lhsT_p1=lhsT_p1, w2stack=w2stack,
        lhsT_g4=lhsT_g4, w1a_d=w1a_d, w1b_lo=w1b_lo, w1b_hi=w1b_hi,
        w1h_d=w1h_d, cvec=cvec,
    )


_SCORE_IDX = None


def _score_index():
    """pair index for each element of the (128, SC_COLS) score output."""
    global _SCORE_IDX
    if _SCORE_IDX is None:
        r = np.arange(128)[:, None]
        col = np.arange(SC_COLS)[None, :]
        j = col // 16
        rem = col % 16
        m, g = rem // 2, rem % 2
        B, cc = j // 21, j % 21
        _SCORE_IDX = (PGCOLS * (2 * B + g) + 1024 * cc + 128 * m + r
                      ).reshape(-1)
    return _SCORE_IDX


def prepare_in_maps(node_feats, edge_feats, hist_tokens, cand_pairs, N,
                    node_W, node_b, edge_W, edge_b,
                    lstm_Wih, lstm_Whh, lstm_bih, lstm_bhh,
                    fuse_W1, fuse_b1, fuse_W2, fuse_b2):
    node_feats = np.asarray(node_feats, np.float32)
    edge_feats = np.asarray(edge_feats, np.float32)
    hist_tokens = np.asarray(hist_tokens, np.float32)
    cand_pairs_in = np.asarray(cand_pairs)
    denom = float(int(N) - 1) + 1e-9

    n_zero_edge = NCORES * (EPAD_ROWS - EPC)
    n_zero_node = NCORES * (NPAD_ROWS - NPC)
    w = _prep_weights(
        np.asarray(node_W), np.asarray(node_b), np.asarray(edge_W),
        np.asarray(edge_b), np.asarray(lstm_Wih), np.asarray(lstm_Whh),
        np.asarray(lstm_bih), np.asarray(lstm_bhh), np.asarray(fuse_W1),
        np.asarray(fuse_b1), np.asarray(fuse_W2), np.asarray(fuse_b2),
        denom, n_zero_node, n_zero_edge,
    )

    in_maps = []
    for c in range(NCORES):
        ebuf = np.zeros((EPAD_ROWS, 5), np.float16)
        ebuf[:EPC] = edge_feats[c * EPC:(c + 1) * EPC]
        e3 = ebuf.reshape(EGROUPS, EGCOLS, 5)       # [group, row, feat]
        edgeT = np.zeros((96, EGCOLS), np.float16)
        for B in range(3):
            gblk = e3[4 * B:4 * B + 4].transpose(0, 2, 1)   # (4, 5, cols)
            edgeT[32 * B:32 * B + 20] = gblk.reshape(20, EGCOLS)

        nbuf = np.zeros((NPAD_ROWS, 2), np.float16)
        nbuf[:NPC] = node_feats[c * NPC:(c + 1) * NPC]
        nodeT = np.ascontiguousarray(
            nbuf.reshape(2, NGCOLS, 2).transpose(0, 2, 1)
        ).reshape(4, NGCOLS)

        pbuf = np.zeros((PPAD, 2), np.float16)
        pbuf[:PPC] = ((cand_pairs_in[c * PPC:(c + 1) * PPC].astype(np.float64)
                       - CAND_C) / CAND_SCALE).astype(np.float16)
        p3 = pbuf.reshape(PGROUPS, PGCOLS, 2)
        candT = np.zeros((12, PGCOLS), np.float16)
        for B in range(3):
            candT[4 * B:4 * B + 4] = (
                p3[2 * B:2 * B + 2].transpose(0, 2, 1).reshape(4, PGCOLS)
            )

        in_maps.append(dict(edgeT=edgeT, nodeT=nodeT, candT=candT,
                            hist=hist_tokens, **w))
    return in_maps


def postprocess(score_arrays):
    idx = _score_index()
    outs = []
    for arr in score_arrays:
        flat = np.empty(PPAD, np.float32)
        flat[idx] = np.asarray(arr).reshape(-1)
        outs.append(flat[:PPC])
    return np.concatenate(outs)


LAST_EXEC_NS = None


def kernel(**inputs):
    global LAST_EXEC_NS
    in_maps = prepare_in_maps(**inputs)
    nc = build_nc()
    trace = bool(os.environ.get("DAD_TRACE"))
    res = run_bass_kernel_spmd(nc, in_maps, list(range(NCORES)), trace=trace)
    LAST_EXEC_NS = res.exec_time_ns
    return postprocess([res.results[c]["scores"] for c in range(NCORES)])

